# revision 12
# baseline (speedup 1.0000x reference)
"""Block-diagonal complex-style locally-connected matmul on 8 NeuronCores.

Math (see reference):
  xp   = x[:, :, perm, :]                  # butterfly permute along N=16384
  xr   = xp[:,0].reshape(B, P, 64)         # P = 4096 blocks, 4*R = 64
  xi   = xp[:,1].reshape(B, P, 64)
  y_re = xr @ W_rr + xi @ W_ri             # per-block [B,64]@[64,64]
  y_im = xr @ W_ir + xi @ W_ii

Device formulation: per block p fold the four 64x64 weights into one
  W_big[p] = [[W_rr, W_ir], [W_ri, W_ii]]  # [128 k, 128 o]
and xcat[b] = [xr|xi]  # [B, 128]; then per block
  y[o, b] = sum_k W_big[k, o] * xcat[b, k]

PE mapping: W_big[p] is the STATIONARY operand (full 128x128 tile, bf16
fast-weight-load) and x of the block [128 k, 8 b] is the moving operand,
so each block costs one LDWEIGHTS (128 cols) + one 8-column matmul and
the output lands PSUM-partition-major: ps[:, blk*8:+8] = y[o, b].
64 blocks fill one PSUM bank [128, 512]; a DVE copy casts the bank to
bf16 in SBUF and one DMA ships it with 1 KB-contiguous descriptors.

All HBM traffic is bf16 (W 16.8 MB, x 1 MB, out 1 MB per core): the
kernel is HBM-bound and bf16 halves the dominant weight stream.  The
host casts inputs to bf16 (rounding adds ~0.1% rms err vs the 2e-2
gate) and restores fp32 on the way out.

Sharding: block axis P=4096 split across 8 cores (512 blocks each).
"""

import sys
import types

import numpy as np
import ml_dtypes

import concourse.bass as bass
import concourse.bacc as bacc
import concourse.tile as tile
from concourse import mybir
from concourse.bass_utils import run_bass_kernel_spmd


def _install_ntff_hook_shim():
    """This image's antenv lacks axon_hooks; rebuild it from the boot helper
    so run_bass_kernel_spmd(trace=True) / BASS_TRACE=1 works instead of
    crashing on the missing module."""
    try:
        from antenv.axon_hooks import get_axon_ntff_profile_hook  # noqa: F401

        return
    except ImportError:
        pass
    try:
        from trn_agent_boot.trn_boot import _ntff_profile_via_ctypes

        hook = _ntff_profile_via_ctypes("/opt/axon/libaxon_pjrt.so")
    except Exception:
        hook = None
    mod = types.ModuleType("antenv.axon_hooks")
    mod.get_axon_ntff_profile_hook = lambda: hook
    mod.set_axon_ntff_profile_hook = lambda h: None
    sys.modules["antenv.axon_hooks"] = mod
    try:
        import antenv

        antenv.axon_hooks = mod
    except ImportError:
        pass


_install_ntff_hook_shim()

B = 8
N = 16384
R = 16
P = 4096            # blocks total
NCORES = 8
PC = P // NCORES    # 512 blocks per core
K = 128             # contraction (4*R re + 4*R im)
O = 128             # output features per block (64 re + 64 im)

CHUNK = 32              # blocks per W-chunk DMA (1 MB bf16)
XSPLIT = 4              # x arrives in 4 slices so matmuls start early

F32 = mybir.dt.float32
BF16 = mybir.dt.bfloat16
NPBF16 = ml_dtypes.bfloat16

_NC_CACHE = None


def _build_bass():
    nc = bacc.Bacc(
        "TRN2", target_bir_lowering=False, debug=False, num_devices=NCORES
    )
    w_dram = nc.declare_dram_parameter("wk", [K, PC * O], BF16, isOutput=False)
    x_dram = nc.declare_dram_parameter("xk", [K, PC * B], BF16, isOutput=False)
    # out[o, blk*8 + b]: o = output feature on the partition axis
    o_dram = nc.declare_dram_parameter("out", [O, PC * B], BF16, isOutput=True)

    n_chunks = PC // CHUNK

    with tile.TileContext(nc) as tc:
        with (
            # bufs == n_chunks: every W DMA enqueues immediately at program
            # start (no buffer reuse), so the SP ring streams W back-to-back
            # with zero mid-stream semaphore stalls.  W stays resident
            # (128 KB/partition of the 208 usable).
            tc.tile_pool(name="wpool", bufs=n_chunks + 2) as wpool,
            tc.tile_pool(name="xpool", bufs=1) as xpool,
            tc.tile_pool(name="stg", bufs=4) as stgpool,
            tc.tile_pool(name="ps", bufs=4, space="PSUM") as pspool,
        ):
            # x + out ride the ACT HWDGE ring so their semaphore waits can't
            # head-of-line block W-chunk descriptor generation on the SP ring.
            x_sb = xpool.tile([K, PC * B], BF16)
            xc = PC * B // XSPLIT
            for xi in range(XSPLIT):
                nc.scalar.dma_start(
                    x_sb[:, xi * xc : (xi + 1) * xc],
                    x_dram[:, xi * xc : (xi + 1) * xc],
                )

            # W chunks and output groups taper at the end: the final W
            # descriptors complete at the slowest SDMA engine's drain time,
            # and everything after that is serial (sem wait -> matmuls ->
            # cast -> store).  Making the last chunk/group 8 blocks cuts
            # that serial chain from ~3.9 us to ~2 us.  Groups are 64-block
            # PSUM banks earlier on (one bank each; PSUM allocation is
            # bank-granular so the DVE cast never reads a bank the PE is
            # still writing).
            w_chunks = [CHUNK] * (n_chunks - 1) + [16, 8, 8]
            groups = [64] * (PC // 64 - 1) + [32, 16, 8, 8]
            gi = 0          # current group index
            goff = 0        # first block of current group
            gdone = 0       # blocks of current group already matmul'd
            ps = None
            blk0 = 0        # first block of current chunk
            for csz in w_chunks:
                w_sb = wpool.tile([K, csz * O], BF16, name="w_sb")
                nc.sync.dma_start(
                    w_sb[:], w_dram[:, blk0 * O : (blk0 + csz) * O]
                )
                for i in range(csz):
                    blk = blk0 + i
                    if gdone == 0:
                        ps = pspool.tile([K, groups[gi] * B], F32, name="ps")
                    nc.tensor.matmul(
                        ps[:, gdone * B : (gdone + 1) * B],
                        w_sb[:, i * O : (i + 1) * O],
                        x_sb[:, blk * B : (blk + 1) * B],
                    )
                    gdone += 1
                    if gdone == groups[gi]:
                        stage = stgpool.tile(
                            [K, groups[gi] * B], BF16, name="stage"
                        )
                        nc.vector.tensor_copy(stage[:], ps[:])
                        nc.scalar.dma_start(
                            o_dram[
                                :, goff * B : (goff + groups[gi]) * B
                            ],
                            stage[:],
                        )
                        goff += groups[gi]
                        gdone = 0
                        gi += 1
                blk0 += csz
    nc.compile()
    return nc


def _get_nc():
    global _NC_CACHE
    if _NC_CACHE is None:
        _NC_CACHE = _build_bass()
    return _NC_CACHE


def _pack_inputs(x, W_rr, W_ri, W_ir, W_ii, perm_idx):
    x = np.asarray(x, dtype=np.float32)
    perm = np.asarray(perm_idx, dtype=np.int64)

    xp = x[:, :, perm, :]                          # [B, 2, N, R]
    xr = xp[:, 0].reshape(B, P, 4 * R)
    xi = xp[:, 1].reshape(B, P, 4 * R)
    xcat = np.concatenate([xr, xi], axis=2)        # [B, P, 128]
    XT = np.ascontiguousarray(xcat.transpose(2, 1, 0))  # [128 k, P, B]
    XT = XT.astype(NPBF16)

    wtop = np.concatenate([W_rr, W_ir], axis=2)    # [P, 64, 128]
    wbot = np.concatenate([W_ri, W_ii], axis=2)    # [P, 64, 128]
    wbig = np.concatenate([wtop, wbot], axis=1)    # [P, 128 k, 128 o]
    WK = np.ascontiguousarray(wbig.transpose(1, 0, 2))  # [128 k, P, 128 o]
    WK = WK.astype(NPBF16)

    in_maps = []
    for c in range(NCORES):
        sl = slice(c * PC, (c + 1) * PC)
        in_maps.append(
            {
                "wk": np.ascontiguousarray(WK[:, sl, :]).reshape(K, PC * O),
                "xk": np.ascontiguousarray(XT[:, sl, :]).reshape(K, PC * B),
            }
        )
    return in_maps


def _unpack_outputs(res):
    ycat = np.empty((B, P, O), dtype=np.float32)   # [b, p, o]
    for c in range(NCORES):
        Oc = np.asarray(res.results[c]["out"]).reshape(O, PC, B)
        ycat[:, c * PC : (c + 1) * PC, :] = Oc.transpose(2, 1, 0).astype(
            np.float32
        )
    y_re = ycat[:, :, : 4 * R].reshape(B, N, R)
    y_im = ycat[:, :, 4 * R :].reshape(B, N, R)
    y = np.stack([y_re, y_im], axis=1)             # [B, 2, N, R]
    return np.ascontiguousarray(y, dtype=np.float32)


def kernel(x, W_rr, W_ri, W_ir, W_ii, perm_idx):
    in_maps = _pack_inputs(x, W_rr, W_ri, W_ir, W_ii, perm_idx)
    nc = _get_nc()
    res = run_bass_kernel_spmd(nc, in_maps, list(range(NCORES)))
    return _unpack_outputs(res)
